# revision 1
# baseline (speedup 1.0000x reference)
"""Causal multi-head self-attention on 8 Trainium2 NeuronCores.

Sharding: tensor-parallel over heads. 16 heads / 8 cores = 2 heads per core.
Each core computes QKV projection for its 2 heads (full sequence, both
batches), causal flash-style attention for its 2 heads, and a partial output
projection against its slice of W_o columns. The host sums the 8 partial
outputs (the "all-reduce" of the tensor-parallel scheme, done during unshard).

Device layout choices (everything keeps the contraction dim on partitions):
  - x is passed pre-transposed as xT [D, B*S] so the QKV projection can
    contract over d_model on the partition axis.
  - Projection computes Q^T/K^T/V^T [128=2*dk, S] per batch directly.
  - Scores are computed transposed, S^T[k, q] = K^T.T @ Q^T, so softmax's
    exp runs on ACT and the AV matmul consumes exp(S^T) directly as rhs.
  - V^T is transposed on the PE (via identity) to V[tok, dv]; a ones column
    is appended so the AV matmul also produces the softmax row-sums.
  - Per-q normalization: DVE reciprocal of the row-sum row, GPSIMD
    partition-broadcast, DVE multiply into mhaT [128, S].
  - Output projection: out[tok,:] = mhaT_tile.T @ WoT with WoT = W_o_slice.T.
Causality: block staircase skips fully-masked blocks; diagonal blocks are
column-sliced and the remaining 128-wide band is masked with a triangular
0/1 mask after exp.
"""

import numpy as np

import concourse.bacc as bacc
import concourse.mybir as mybir
import concourse.tile as tile

FP32 = mybir.dt.float32

B = 2
S = 2048
D = 1024
NUM_HEADS = 16
DK = 64
NCORES = 8
HPC = NUM_HEADS // NCORES  # heads per core = 2
HD = HPC * DK  # 128, head dims per core

QCW = 512  # q chunk width (matmul moving-dim limit for fp32)
KTW = 128  # k tile width (partition dim)


def build_nc(d=D, s=S, b=B):
    """Build the per-core Bass program. All 8 cores run this same program."""
    assert d % 128 == 0 and s % QCW == 0 and QCW % KTW == 0
    ndc = d // 128  # d_model chunks
    nqc = s // QCW  # q chunks per batch
    nkt = s // KTW  # k tiles per batch
    kpq = QCW // KTW  # k tiles per q chunk (4)

    nc = bacc.Bacc("TRN2", target_bir_lowering=False)

    xT_d = nc.dram_tensor("xT", [d, b * s], FP32, kind="ExternalInput")
    wt_d = nc.dram_tensor("wqkvT", [d, 3 * HD], FP32, kind="ExternalInput")
    wo_d = nc.dram_tensor("woT", [HD, d], FP32, kind="ExternalInput")
    tri_d = nc.dram_tensor("tri", [128, 128], FP32, kind="ExternalInput")
    id_d = nc.dram_tensor("ident", [128, 128], FP32, kind="ExternalInput")
    out_d = nc.dram_tensor("out", [b * s, d], FP32, kind="ExternalOutput")

    with tile.TileContext(nc) as tc:
        with (
            tc.tile_pool(name="consts", bufs=1) as consts,
            tc.tile_pool(name="xts", bufs=ndc) as xts_pool,
            tc.tile_pool(name="qkv", bufs=1) as qkv_pool,
            tc.tile_pool(name="vsb", bufs=1) as v_pool,
            tc.tile_pool(name="pt", bufs=3) as pt_pool,
            tc.tile_pool(name="mha", bufs=1) as mha_pool,
            tc.tile_pool(name="osb", bufs=3) as out_pool,
            tc.tile_pool(name="small", bufs=4) as small_pool,
            tc.tile_pool(name="ps_s", bufs=2, space="PSUM") as ps_s,
            tc.tile_pool(name="ps_o", bufs=1, space="PSUM") as ps_o,
            tc.tile_pool(name="ps_aux", bufs=2, space="PSUM") as ps_aux,
        ):
            wt_sb = consts.tile([128, ndc, 3 * HD], FP32)
            for k in range(ndc):
                nc.sync.dma_start(wt_sb[:, k, :], wt_d[128 * k : 128 * (k + 1), :])
            wo_sb = consts.tile([128, d], FP32)
            nc.sync.dma_start(wo_sb, wo_d[:, :])
            tri_sb = consts.tile([128, 128], FP32)
            nc.sync.dma_start(tri_sb, tri_d[:, :])
            id_sb = consts.tile([128, 128], FP32)
            nc.sync.dma_start(id_sb, id_d[:, :])

            for bi in range(b):
                # ---- load x^T for this batch ----
                xts = []
                for k in range(ndc):
                    xt = xts_pool.tile([128, s], FP32, name=f"xt{k}", tag="xt")
                    nc.sync.dma_start(
                        xt, xT_d[128 * k : 128 * (k + 1), bi * s : (bi + 1) * s]
                    )
                    xts.append(xt)

                # ---- QKV projection: qkvT[:, m, :] = (W_m x_b^T) ----
                qkvT = qkv_pool.tile([128, 3, s], FP32, tag="qkvT")
                for m in range(3):
                    for n in range(nqc):
                        pp = ps_aux.tile([128, QCW], FP32, name="pp", tag="aux")
                        for k in range(ndc):
                            nc.tensor.matmul(
                                pp,
                                wt_sb[:, k, 128 * m : 128 * (m + 1)],
                                xts[k][:, QCW * n : QCW * (n + 1)],
                                start=(k == 0),
                                stop=(k == ndc - 1),
                            )
                        nc.vector.tensor_copy(qkvT[:, m, QCW * n : QCW * (n + 1)], pp)

                # ---- V^T -> V[tok, dv] with ones column appended ----
                v_sb = v_pool.tile([128, nkt, 2 * (DK + 1)], FP32, tag="vsb")
                nc.gpsimd.memset(v_sb, 1.0)
                for t in range(nkt):
                    tp = ps_aux.tile([128, 128], FP32, name="tp", tag="aux")
                    nc.tensor.transpose(
                        tp, qkvT[:, 2, 128 * t : 128 * (t + 1)], id_sb
                    )
                    nc.vector.tensor_copy(v_sb[:, t, 0:DK], tp[:, 0:DK])
                    nc.vector.tensor_copy(
                        v_sb[:, t, DK + 1 : 2 * DK + 1], tp[:, DK : 2 * DK]
                    )

                # ---- attention (both heads together) ----
                mhaT = mha_pool.tile([128, s], FP32, tag="mhaT")
                for qc in range(nqc):
                    q0 = QCW * qc
                    oA = ps_o.tile([DK + 1, QCW], FP32, name="oA", tag="oA")
                    oB = ps_o.tile([DK + 1, QCW], FP32, name="oB", tag="oB")
                    kts = kpq * (qc + 1)
                    for kt in range(kts):
                        c0 = KTW * (kt - kpq * qc) if kt >= kpq * qc else 0
                        sp = ps_s.tile([128, 2, QCW], FP32, name="sp", tag="s")
                        # scores S^T[k, q] for head A (partitions 0:64) and
                        # head B (partitions 64:128) -> separate PE row groups
                        nc.tensor.matmul(
                            sp[:, 0, c0:QCW],
                            qkvT[0:DK, 1, KTW * kt : KTW * (kt + 1)],
                            qkvT[0:DK, 0, q0 + c0 : q0 + QCW],
                        )
                        nc.tensor.matmul(
                            sp[:, 1, c0:QCW],
                            qkvT[DK : 2 * DK, 1, KTW * kt : KTW * (kt + 1)],
                            qkvT[DK : 2 * DK, 0, q0 + c0 : q0 + QCW],
                        )
                        pt = pt_pool.tile([128, 2, QCW], FP32, name="pt", tag="pt")
                        nc.scalar.activation(
                            pt[:, :, c0:QCW],
                            sp[:, :, c0:QCW],
                            mybir.ActivationFunctionType.Exp,
                        )
                        if kt >= kpq * qc:
                            # triangular mask on the diagonal 128-band
                            nc.vector.tensor_mul(
                                pt[:, 0, c0 : c0 + KTW],
                                pt[:, 0, c0 : c0 + KTW],
                                tri_sb,
                            )
                            nc.vector.tensor_mul(
                                pt[:, 1, c0 : c0 + KTW],
                                pt[:, 1, c0 : c0 + KTW],
                                tri_sb,
                            )
                        nc.tensor.matmul(
                            oA[:, c0:QCW],
                            v_sb[:, kt, 0 : DK + 1],
                            pt[:, 0, c0:QCW],
                            start=(kt == 0),
                            stop=(kt == kts - 1),
                        )
                        nc.tensor.matmul(
                            oB[:, c0:QCW],
                            v_sb[:, kt, DK + 1 : 2 * DK + 2],
                            pt[:, 1, c0:QCW],
                            start=(kt == 0),
                            stop=(kt == kts - 1),
                        )
                    # normalize: rows 0:64 are O^T, row 64 is the softmax sum
                    rcA = small_pool.tile([1, QCW], FP32, name="rcA", tag="rcA")
                    rcB = small_pool.tile([1, QCW], FP32, name="rcB", tag="rcB")
                    nc.vector.reciprocal(rcA, oA[DK : DK + 1, :])
                    nc.vector.reciprocal(rcB, oB[DK : DK + 1, :])
                    bcA = small_pool.tile([DK, QCW], FP32, name="bcA", tag="bcA")
                    bcB = small_pool.tile([DK, QCW], FP32, name="bcB", tag="bcB")
                    nc.gpsimd.partition_broadcast(bcA, rcA, channels=DK)
                    nc.gpsimd.partition_broadcast(bcB, rcB, channels=DK)
                    nc.vector.tensor_mul(
                        mhaT[0:DK, q0 : q0 + QCW], oA[0:DK, :], bcA
                    )
                    nc.vector.tensor_mul(
                        mhaT[DK : 2 * DK, q0 : q0 + QCW], oB[0:DK, :], bcB
                    )

                # ---- output projection (partial over this core's heads) ----
                for t in range(s // 128):
                    fps = []
                    for half in range(d // QCW):
                        fp = ps_aux.tile([128, QCW], FP32, name="fp", tag="aux")
                        nc.tensor.matmul(
                            fp,
                            mhaT[:, 128 * t : 128 * (t + 1)],
                            wo_sb[:, QCW * half : QCW * (half + 1)],
                        )
                        fps.append(fp)
                    ob = out_pool.tile([128, d], FP32, name="ob", tag="ob")
                    for half in range(d // QCW):
                        nc.vector.tensor_copy(
                            ob[:, QCW * half : QCW * (half + 1)], fps[half]
                        )
                    r0 = bi * s + 128 * t
                    nc.sync.dma_start(out_d[r0 : r0 + 128, :], ob)

    nc.compile()
    return nc


def make_core_inputs(x, W_qkv, W_o, d=D, s=S, b=B):
    """Host-side shard prep. Returns list of per-core input dicts."""
    nh = W_qkv.shape[0] // (3 * DK)
    xT = np.ascontiguousarray(
        x.astype(np.float32).transpose(2, 0, 1).reshape(d, b * s)
    )
    tri = np.triu(np.ones((128, 128), dtype=np.float32))  # tri[k,q]=1 iff q>=k
    ident = np.eye(128, dtype=np.float32)
    scale = np.float32(1.0 / np.sqrt(DK))
    in_maps = []
    for c in range(NCORES):
        h0 = HPC * c
        r = slice(h0 * DK, (h0 + HPC) * DK)
        wq = W_qkv[0 * nh * DK :][r] * scale
        wk = W_qkv[1 * nh * DK :][r]
        wv = W_qkv[2 * nh * DK :][r]
        ws = np.concatenate([wq, wk, wv], axis=0)  # [3*HD, d]
        wT = np.ascontiguousarray(ws.T.astype(np.float32))  # [d, 3*HD]
        woT = np.ascontiguousarray(W_o[:, r].T.astype(np.float32))  # [HD, d]
        in_maps.append(
            {"xT": xT, "wqkvT": wT, "woT": woT, "tri": tri, "ident": ident}
        )
    return in_maps


_NC_CACHE = {}


def kernel(x, W_qkv, W_o):
    from concourse.bass_utils import run_bass_kernel_spmd

    b, s, d = x.shape
    if "nc" not in _NC_CACHE:
        _NC_CACHE["nc"] = build_nc(d=d, s=s, b=b)
    nc = _NC_CACHE["nc"]
    in_maps = make_core_inputs(x, W_qkv, W_o, d=d, s=s, b=b)
    res = run_bass_kernel_spmd(nc, in_maps, core_ids=list(range(NCORES)))
    out = res.results[0]["out"].astype(np.float64)
    for c in range(1, NCORES):
        out += res.results[c]["out"]
    return out.astype(np.float32).reshape(b, s, d)


# revision 10
# speedup vs baseline: 3.0054x; 3.0054x over previous
"""Causal multi-head self-attention on 8 Trainium2 NeuronCores.

Sharding: tensor-parallel over heads. 16 heads / 8 cores = 2 heads per core.
Each core computes the QKV projection for its 2 heads (full sequence, both
batches), causal flash-style attention for its 2 heads, and a partial output
projection against its slice of W_o columns. The host sums the 8 partial
outputs (the "all-reduce" of the tensor-parallel scheme, done during unshard).

Matmul inputs are fp16 (PE streams 1 row/cycle vs 4 for fp32; fp16 keeps
11 mantissa bits vs bf16's 8), accumulation is always fp32 in PSUM, softmax
runs in fp32. End-to-end error vs the fp32 reference is ~4e-4 relative.

Device layout (contraction dim always on partitions):
  - x passed pre-transposed and pre-cast: xT [D, B*S] fp16.
  - Projection computes Q^T/K^T/V^T [128=2*dk, S] per batch directly.
  - Scores computed transposed, S^T[k, q] = K^T.T @ Q^T (fp32 PSUM), both
    heads into one [128, 2, 512] PSUM tile via separate PE row groups.
  - One ACT exp per score tile (PSUM -> SBUF fp16), causal diagonal blocks
    column-sliced, the remaining 128-band masked with a triangular multiply.
  - V^T transposed on-PE to V[tok, dv] with a ones column appended, so the
    AV matmul also accumulates the softmax row-sums (row 64 of the output).
  - Normalization: batched ~2ULP reciprocal of the 8 row-sum rows per batch,
    GPSIMD partition-broadcast, DVE multiply into mhaT fp16.
  - Output projection: out[tok,:] = mhaT_tile.T @ WoT, fp32 result to DRAM.
"""

import numpy as np

import concourse.bacc as bacc
import concourse.mybir as mybir
import concourse.tile as tile

FP32 = mybir.dt.float32
FP16 = mybir.dt.float16

B = 2
S = 2048
D = 1024
NUM_HEADS = 16
DK = 64
NCORES = 8
HPC = NUM_HEADS // NCORES  # heads per core = 2
HD = HPC * DK  # 128, head dims per core

QCW = 512  # q chunk width
KTW = 128  # k tile width (partition dim)

NP_IN = np.float16


def build_nc(d=D, s=S, b=B):
    """Build the per-core Bass program. All 8 cores run this same program."""
    assert d % 128 == 0 and s % QCW == 0 and QCW % KTW == 0
    ndc = d // 128  # d_model chunks
    nqc = s // QCW  # q chunks per batch
    nkt = s // KTW  # k tiles per batch
    kpq = QCW // KTW  # k tiles per q chunk (4)

    nc = bacc.Bacc("TRN2", target_bir_lowering=False)

    xT_d = nc.dram_tensor("xT", [d, b * s], FP16, kind="ExternalInput")
    wt_d = nc.dram_tensor("wqkvT", [d, 3 * HD], FP16, kind="ExternalInput")
    wo_d = nc.dram_tensor("woT", [HD, d], FP16, kind="ExternalInput")
    tri_d = nc.dram_tensor("tri", [128, 128], FP16, kind="ExternalInput")
    id_d = nc.dram_tensor("ident", [128, 128], FP16, kind="ExternalInput")
    out_d = nc.dram_tensor("out", [b * s, d], FP32, kind="ExternalOutput")

    with tile.TileContext(nc) as tc:
        with (
            tc.tile_pool(name="consts", bufs=1) as consts,
            tc.tile_pool(name="xts", bufs=2 * ndc) as xts_pool,
            tc.tile_pool(name="qkv", bufs=2) as qkv_pool,
            tc.tile_pool(name="vsb", bufs=2) as v_pool,
            tc.tile_pool(name="pt", bufs=3) as pt_pool,
            tc.tile_pool(name="mha", bufs=2) as mha_pool,
            tc.tile_pool(name="osb", bufs=3) as out_pool,
            tc.tile_pool(name="small", bufs=2) as small_pool,
            tc.tile_pool(name="ps_s", bufs=2, space="PSUM") as ps_s,
            tc.tile_pool(name="ps_o", bufs=1, space="PSUM") as ps_o,
            tc.tile_pool(name="ps_aux", bufs=2, space="PSUM") as ps_aux,
        ):
            wt_sb = consts.tile([128, ndc, 3 * HD], FP16)
            for k in range(ndc):
                nc.sync.dma_start(wt_sb[:, k, :], wt_d[128 * k : 128 * (k + 1), :])
            wo_sb = consts.tile([128, d], FP16)
            nc.sync.dma_start(wo_sb, wo_d[:, :])
            tri_sb = consts.tile([128, 128], FP16)
            nc.sync.dma_start(tri_sb, tri_d[:, :])
            id_sb = consts.tile([128, 128], FP16)
            nc.sync.dma_start(id_sb, id_d[:, :])

            for bi in range(b):
                # ---- load x^T for this batch ----
                xts = []
                for k in range(ndc):
                    xt = xts_pool.tile([128, s], FP16, name=f"xt{k}", tag="xt")
                    nc.sync.dma_start(
                        xt, xT_d[128 * k : 128 * (k + 1), bi * s : (bi + 1) * s]
                    )
                    xts.append(xt)

                # ---- QKV projection: qkvT[:, m, :] = (W_m x_b^T) ----
                qkvT = qkv_pool.tile([128, 3, s], FP16, tag="qkvT")
                for m in range(3):
                    for n in range(nqc):
                        pp = ps_aux.tile([128, QCW], FP32, name="pp", tag="aux")
                        for k in range(ndc):
                            nc.tensor.matmul(
                                pp,
                                wt_sb[:, k, 128 * m : 128 * (m + 1)],
                                xts[k][:, QCW * n : QCW * (n + 1)],
                                start=(k == 0),
                                stop=(k == ndc - 1),
                            )
                        # ACT is idle during the projection phase; it also
                        # casts fp32 PSUM -> fp16 SBUF on the way out.
                        nc.scalar.copy(qkvT[:, m, QCW * n : QCW * (n + 1)], pp)

                # ---- V^T -> V[tok, dv] with ones column appended ----
                v_sb = v_pool.tile([128, nkt, 2 * (DK + 1)], FP16, tag="vsb")
                nc.gpsimd.memset(v_sb, 1.0)
                for t in range(nkt):
                    tp = ps_aux.tile([128, 128], FP16, name="tp", tag="aux")
                    nc.tensor.transpose(
                        tp, qkvT[:, 2, 128 * t : 128 * (t + 1)], id_sb
                    )
                    nc.vector.tensor_copy(v_sb[:, t, 0:DK], tp[:, 0:DK])
                    nc.vector.tensor_copy(
                        v_sb[:, t, DK + 1 : 2 * DK + 1], tp[:, DK : 2 * DK]
                    )

                # ---- attention (both heads together) ----
                mhaT = mha_pool.tile([128, s], FP16, tag="mhaT")
                for qc in range(nqc):
                    q0 = QCW * qc
                    oA = ps_o.tile([DK + 1, QCW], FP32, name="oA", tag="oA")
                    oB = ps_o.tile([DK + 1, QCW], FP32, name="oB", tag="oB")
                    kts = kpq * (qc + 1)
                    for kt in range(kts):
                        c0 = KTW * (kt - kpq * qc) if kt >= kpq * qc else 0
                        sp = ps_s.tile([128, 2, QCW], FP32, name="sp", tag="s")
                        # scores S^T[k, q]; head A rows 0:64, head B 64:128
                        nc.tensor.matmul(
                            sp[:, 0, c0:QCW],
                            qkvT[0:DK, 1, KTW * kt : KTW * (kt + 1)],
                            qkvT[0:DK, 0, q0 + c0 : q0 + QCW],
                        )
                        nc.tensor.matmul(
                            sp[:, 1, c0:QCW],
                            qkvT[DK : 2 * DK, 1, KTW * kt : KTW * (kt + 1)],
                            qkvT[DK : 2 * DK, 0, q0 + c0 : q0 + QCW],
                        )
                        pt = pt_pool.tile([128, 2, QCW], FP16, name="pt", tag="pt")
                        nc.scalar.activation(
                            pt[:, :, c0:QCW],
                            sp[:, :, c0:QCW],
                            mybir.ActivationFunctionType.Exp,
                        )
                        if kt >= kpq * qc:
                            # triangular mask on the diagonal 128-band
                            nc.vector.tensor_mul(
                                pt[:, 0, c0 : c0 + KTW],
                                pt[:, 0, c0 : c0 + KTW],
                                tri_sb,
                            )
                            nc.vector.tensor_mul(
                                pt[:, 1, c0 : c0 + KTW],
                                pt[:, 1, c0 : c0 + KTW],
                                tri_sb,
                            )
                        nc.tensor.matmul(
                            oA[:, c0:QCW],
                            v_sb[:, kt, 0 : DK + 1],
                            pt[:, 0, c0:QCW],
                            start=(kt == 0),
                            stop=(kt == kts - 1),
                        )
                        nc.tensor.matmul(
                            oB[:, c0:QCW],
                            v_sb[:, kt, DK + 1 : 2 * DK + 2],
                            pt[:, 1, c0:QCW],
                            start=(kt == 0),
                            stop=(kt == kts - 1),
                        )
                    # normalize: per head, broadcast the row-sum row (base-0
                    # output only -- HW partition_broadcast ignores out base),
                    # reciprocal it, multiply O^T (PSUM) into mhaT (fp16)
                    for h, oh in ((0, oA), (1, oB)):
                        t = small_pool.tile([1, QCW], FP32, name="t", tag=f"t{h}")
                        nc.vector.tensor_copy(t, oh[DK : DK + 1, :])
                        bc = small_pool.tile(
                            [DK, QCW], FP32, name="bc", tag=f"bc{h}"
                        )
                        nc.gpsimd.partition_broadcast(bc, t, channels=DK)
                        nc.vector.reciprocal_approx_fast(out=bc, in_=bc)
                        nc.vector.tensor_mul(
                            mhaT[DK * h : DK * (h + 1), q0 : q0 + QCW],
                            oh[0:DK, :],
                            bc,
                        )

                # ---- output projection (partial over this core's heads) ----
                for t in range(s // 128):
                    fps = []
                    for half in range(d // QCW):
                        fp = ps_aux.tile([128, QCW], FP32, name="fp", tag="aux")
                        nc.tensor.matmul(
                            fp,
                            mhaT[:, 128 * t : 128 * (t + 1)],
                            wo_sb[:, QCW * half : QCW * (half + 1)],
                        )
                        fps.append(fp)
                    ob = out_pool.tile([128, d], FP32, name="ob", tag="ob")
                    for half in range(d // QCW):
                        nc.vector.tensor_copy(
                            ob[:, QCW * half : QCW * (half + 1)], fps[half]
                        )
                    r0 = bi * s + 128 * t
                    nc.sync.dma_start(out_d[r0 : r0 + 128, :], ob)

    nc.compile()
    return nc


def make_core_inputs(x, W_qkv, W_o, d=D, s=S, b=B):
    """Host-side shard prep. Returns list of per-core input dicts."""
    nh = W_qkv.shape[0] // (3 * DK)
    xT = np.ascontiguousarray(
        x.astype(np.float32).transpose(2, 0, 1).reshape(d, b * s).astype(NP_IN)
    )
    tri = np.triu(np.ones((128, 128), dtype=NP_IN))  # tri[k,q]=1 iff q>=k
    ident = np.eye(128, dtype=NP_IN)
    scale = np.float32(1.0 / np.sqrt(DK))
    in_maps = []
    for c in range(NCORES):
        h0 = HPC * c
        r = slice(h0 * DK, (h0 + HPC) * DK)
        wq = W_qkv[0 * nh * DK :][r] * scale
        wk = W_qkv[1 * nh * DK :][r]
        wv = W_qkv[2 * nh * DK :][r]
        ws = np.concatenate([wq, wk, wv], axis=0)  # [3*HD, d]
        wT = np.ascontiguousarray(ws.T.astype(NP_IN))  # [d, 3*HD]
        woT = np.ascontiguousarray(W_o[:, r].T.astype(NP_IN))  # [HD, d]
        in_maps.append(
            {"xT": xT, "wqkvT": wT, "woT": woT, "tri": tri, "ident": ident}
        )
    return in_maps


_NC_CACHE = {}


def kernel(x, W_qkv, W_o):
    from concourse.bass_utils import run_bass_kernel_spmd

    b, s, d = x.shape
    if "nc" not in _NC_CACHE:
        _NC_CACHE["nc"] = build_nc(d=d, s=s, b=b)
    nc = _NC_CACHE["nc"]
    in_maps = make_core_inputs(x, W_qkv, W_o, d=d, s=s, b=b)
    res = run_bass_kernel_spmd(nc, in_maps, core_ids=list(range(NCORES)))
    out = res.results[0]["out"].astype(np.float64)
    for c in range(1, NCORES):
        out += res.results[c]["out"]
    return out.astype(np.float32).reshape(b, s, d)


# revision 12
# speedup vs baseline: 3.1312x; 1.0418x over previous
"""Causal multi-head self-attention on 8 Trainium2 NeuronCores.

Sharding: tensor-parallel over heads. 16 heads / 8 cores = 2 heads per core.
Each core computes the QKV projection for its 2 heads (full sequence, both
batches), causal flash-style attention for its 2 heads, and a partial output
projection against its slice of W_o columns. The host sums the 8 partial
outputs (the "all-reduce" of the tensor-parallel scheme, done during unshard).

Matmul inputs are fp16 (PE streams 1 row/cycle vs 4 for fp32; fp16 keeps
11 mantissa bits vs bf16's 8), accumulation is always fp32 in PSUM, softmax
runs in fp32. End-to-end error vs the fp32 reference is ~4e-4 relative.

Device layout (contraction dim always on partitions):
  - x passed pre-transposed and pre-cast: xT [D, B*S] fp16.
  - Projection computes Q^T/K^T/V^T [128=2*dk, S] per batch directly.
  - Scores computed transposed, S^T[k, q] = K^T.T @ Q^T (fp32 PSUM), both
    heads into one [128, 2, 512] PSUM tile via separate PE row groups.
  - One ACT exp per score tile (PSUM -> SBUF fp16), causal diagonal blocks
    column-sliced, the remaining 128-band masked with a triangular multiply.
  - V^T transposed on-PE to V[tok, dv] with a ones column appended, so the
    AV matmul also accumulates the softmax row-sums (row 64 of the output).
  - Normalization: batched ~2ULP reciprocal of the 8 row-sum rows per batch,
    GPSIMD partition-broadcast, DVE multiply into mhaT fp16.
  - Output projection: out[tok,:] = mhaT_tile.T @ WoT, fp32 result to DRAM.
"""

import numpy as np

import concourse.bacc as bacc
import concourse.mybir as mybir
import concourse.tile as tile

FP32 = mybir.dt.float32
FP16 = mybir.dt.float16

B = 2
S = 2048
D = 1024
NUM_HEADS = 16
DK = 64
NCORES = 8
HPC = NUM_HEADS // NCORES  # heads per core = 2
HD = HPC * DK  # 128, head dims per core

QCW = 512  # q chunk width
KTW = 128  # k tile width (partition dim)

NP_IN = np.float16


def build_nc(d=D, s=S, b=B):
    """Build the per-core Bass program. All 8 cores run this same program."""
    assert d % 128 == 0 and s % QCW == 0 and QCW % KTW == 0
    ndc = d // 128  # d_model chunks
    nqc = s // QCW  # q chunks per batch
    nkt = s // KTW  # k tiles per batch
    kpq = QCW // KTW  # k tiles per q chunk (4)

    nc = bacc.Bacc("TRN2", target_bir_lowering=False)

    xT_d = nc.dram_tensor("xT", [d, b * s], FP16, kind="ExternalInput")
    wt_d = nc.dram_tensor("wqkvT", [d, 3 * HD], FP16, kind="ExternalInput")
    wo_d = nc.dram_tensor("woT", [HD, d], FP16, kind="ExternalInput")
    tri_d = nc.dram_tensor("tri", [128, 128], FP16, kind="ExternalInput")
    id_d = nc.dram_tensor("ident", [128, 128], FP16, kind="ExternalInput")
    out_d = nc.dram_tensor("out", [b * s, d], FP32, kind="ExternalOutput")

    with tile.TileContext(nc) as tc:
        with (
            tc.tile_pool(name="consts", bufs=1) as consts,
            tc.tile_pool(name="xts", bufs=2 * ndc) as xts_pool,
            tc.tile_pool(name="qkv", bufs=2) as qkv_pool,
            tc.tile_pool(name="vsb", bufs=2) as v_pool,
            tc.tile_pool(name="pt", bufs=3) as pt_pool,
            tc.tile_pool(name="mha", bufs=2) as mha_pool,
            tc.tile_pool(name="osb", bufs=3) as out_pool,
            tc.tile_pool(name="small", bufs=2) as small_pool,
            tc.tile_pool(name="ps_s", bufs=2, space="PSUM") as ps_s,
            tc.tile_pool(name="ps_o", bufs=1, space="PSUM") as ps_o,
            tc.tile_pool(name="ps_aux", bufs=2, space="PSUM") as ps_aux,
        ):
            wt_sb = consts.tile([128, ndc, 3 * HD], FP16)
            for k in range(ndc):
                nc.sync.dma_start(wt_sb[:, k, :], wt_d[128 * k : 128 * (k + 1), :])
            wo_sb = consts.tile([128, d], FP16)
            nc.sync.dma_start(wo_sb, wo_d[:, :])
            tri_sb = consts.tile([128, 128], FP16)
            nc.sync.dma_start(tri_sb, tri_d[:, :])
            id_sb = consts.tile([128, 128], FP16)
            nc.sync.dma_start(id_sb, id_d[:, :])

            # ---- load x^T for both batches up front (prefetch) ----
            xts_all = []
            for bi in range(b):
                xts = []
                for k in range(ndc):
                    xt = xts_pool.tile([128, s], FP16, name=f"xt{bi}_{k}", tag="xt")
                    nc.sync.dma_start(
                        xt, xT_d[128 * k : 128 * (k + 1), bi * s : (bi + 1) * s]
                    )
                    xts.append(xt)
                xts_all.append(xts)

            for bi in range(b):
                xts = xts_all[bi]
                # ---- QKV projection: qkvT[:, m, :] = (W_m x_b^T) ----
                qkvT = qkv_pool.tile([128, 3, s], FP16, tag="qkvT")
                for m in range(3):
                    for n in range(nqc):
                        pp = ps_aux.tile([128, QCW], FP32, name="pp", tag="aux")
                        for k in range(ndc):
                            nc.tensor.matmul(
                                pp,
                                wt_sb[:, k, 128 * m : 128 * (m + 1)],
                                xts[k][:, QCW * n : QCW * (n + 1)],
                                start=(k == 0),
                                stop=(k == ndc - 1),
                            )
                        # ACT is idle during the projection phase; it also
                        # casts fp32 PSUM -> fp16 SBUF on the way out.
                        nc.scalar.copy(qkvT[:, m, QCW * n : QCW * (n + 1)], pp)

                # ---- V^T -> V[tok, dv] with ones column appended ----
                v_sb = v_pool.tile([128, nkt, 2 * (DK + 1)], FP16, tag="vsb")
                nc.gpsimd.memset(v_sb, 1.0)
                for t in range(nkt):
                    tp = ps_aux.tile([128, 128], FP16, name="tp", tag="aux")
                    nc.tensor.transpose(
                        tp, qkvT[:, 2, 128 * t : 128 * (t + 1)], id_sb
                    )
                    nc.vector.tensor_copy(v_sb[:, t, 0:DK], tp[:, 0:DK])
                    nc.vector.tensor_copy(
                        v_sb[:, t, DK + 1 : 2 * DK + 1], tp[:, DK : 2 * DK]
                    )

                # ---- attention (both heads together) ----
                mhaT = mha_pool.tile([128, s], FP16, tag="mhaT")
                for qc in range(nqc):
                    q0 = QCW * qc
                    oA = ps_o.tile([DK + 1, QCW], FP32, name="oA", tag="oA")
                    oB = ps_o.tile([DK + 1, QCW], FP32, name="oB", tag="oB")
                    kts = kpq * (qc + 1)
                    for kt in range(kts):
                        c0 = KTW * (kt - kpq * qc) if kt >= kpq * qc else 0
                        sp = ps_s.tile([128, 2, QCW], FP32, name="sp", tag="s")
                        # scores S^T[k, q]; head A rows 0:64, head B 64:128
                        nc.tensor.matmul(
                            sp[:, 0, c0:QCW],
                            qkvT[0:DK, 1, KTW * kt : KTW * (kt + 1)],
                            qkvT[0:DK, 0, q0 + c0 : q0 + QCW],
                        )
                        nc.tensor.matmul(
                            sp[:, 1, c0:QCW],
                            qkvT[DK : 2 * DK, 1, KTW * kt : KTW * (kt + 1)],
                            qkvT[DK : 2 * DK, 0, q0 + c0 : q0 + QCW],
                        )
                        pt = pt_pool.tile([128, 2, QCW], FP16, name="pt", tag="pt")
                        nc.scalar.activation(
                            pt[:, :, c0:QCW],
                            sp[:, :, c0:QCW],
                            mybir.ActivationFunctionType.Exp,
                        )
                        if kt >= kpq * qc:
                            # triangular mask on the diagonal 128-band
                            nc.vector.tensor_mul(
                                pt[:, 0, c0 : c0 + KTW],
                                pt[:, 0, c0 : c0 + KTW],
                                tri_sb,
                            )
                            nc.vector.tensor_mul(
                                pt[:, 1, c0 : c0 + KTW],
                                pt[:, 1, c0 : c0 + KTW],
                                tri_sb,
                            )
                        nc.tensor.matmul(
                            oA[:, c0:QCW],
                            v_sb[:, kt, 0 : DK + 1],
                            pt[:, 0, c0:QCW],
                            start=(kt == 0),
                            stop=(kt == kts - 1),
                        )
                        nc.tensor.matmul(
                            oB[:, c0:QCW],
                            v_sb[:, kt, DK + 1 : 2 * DK + 2],
                            pt[:, 1, c0:QCW],
                            start=(kt == 0),
                            stop=(kt == kts - 1),
                        )
                    # normalize: stage O^T and row-sum to base-0 SBUF tiles
                    # (frees the PSUM accumulators for the next chunk), then
                    # broadcast the row-sum (base-0 output only -- HW
                    # partition_broadcast ignores out base), reciprocal,
                    # multiply into mhaT (fp16)
                    for h, oh in ((0, oA), (1, oB)):
                        ost = small_pool.tile(
                            [DK, QCW], FP32, name="ost", tag=f"ost{h}"
                        )
                        nc.vector.tensor_copy(ost, oh[0:DK, :])
                        t = small_pool.tile([1, QCW], FP32, name="t", tag=f"t{h}")
                        nc.vector.tensor_copy(t, oh[DK : DK + 1, :])
                        bc = small_pool.tile(
                            [DK, QCW], FP32, name="bc", tag=f"bc{h}"
                        )
                        nc.gpsimd.partition_broadcast(bc, t, channels=DK)
                        nc.vector.reciprocal_approx_fast(out=bc, in_=bc)
                        nc.vector.tensor_mul(
                            mhaT[DK * h : DK * (h + 1), q0 : q0 + QCW],
                            ost,
                            bc,
                        )

                # ---- output projection (partial over this core's heads) ----
                for t in range(s // 128):
                    fps = []
                    for half in range(d // QCW):
                        fp = ps_aux.tile([128, QCW], FP32, name="fp", tag="aux")
                        nc.tensor.matmul(
                            fp,
                            mhaT[:, 128 * t : 128 * (t + 1)],
                            wo_sb[:, QCW * half : QCW * (half + 1)],
                        )
                        fps.append(fp)
                    ob = out_pool.tile([128, d], FP32, name="ob", tag="ob")
                    for half in range(d // QCW):
                        nc.vector.tensor_copy(
                            ob[:, QCW * half : QCW * (half + 1)], fps[half]
                        )
                    r0 = bi * s + 128 * t
                    nc.sync.dma_start(out_d[r0 : r0 + 128, :], ob)

    nc.compile()
    return nc


def make_core_inputs(x, W_qkv, W_o, d=D, s=S, b=B):
    """Host-side shard prep. Returns list of per-core input dicts."""
    nh = W_qkv.shape[0] // (3 * DK)
    xT = np.ascontiguousarray(
        x.astype(np.float32).transpose(2, 0, 1).reshape(d, b * s).astype(NP_IN)
    )
    tri = np.triu(np.ones((128, 128), dtype=NP_IN))  # tri[k,q]=1 iff q>=k
    ident = np.eye(128, dtype=NP_IN)
    scale = np.float32(1.0 / np.sqrt(DK))
    in_maps = []
    for c in range(NCORES):
        h0 = HPC * c
        r = slice(h0 * DK, (h0 + HPC) * DK)
        wq = W_qkv[0 * nh * DK :][r] * scale
        wk = W_qkv[1 * nh * DK :][r]
        wv = W_qkv[2 * nh * DK :][r]
        ws = np.concatenate([wq, wk, wv], axis=0)  # [3*HD, d]
        wT = np.ascontiguousarray(ws.T.astype(NP_IN))  # [d, 3*HD]
        woT = np.ascontiguousarray(W_o[:, r].T.astype(NP_IN))  # [HD, d]
        in_maps.append(
            {"xT": xT, "wqkvT": wT, "woT": woT, "tri": tri, "ident": ident}
        )
    return in_maps


_NC_CACHE = {}


def kernel(x, W_qkv, W_o):
    from concourse.bass_utils import run_bass_kernel_spmd

    b, s, d = x.shape
    if "nc" not in _NC_CACHE:
        _NC_CACHE["nc"] = build_nc(d=d, s=s, b=b)
    nc = _NC_CACHE["nc"]
    in_maps = make_core_inputs(x, W_qkv, W_o, d=d, s=s, b=b)
    res = run_bass_kernel_spmd(nc, in_maps, core_ids=list(range(NCORES)))
    out = res.results[0]["out"].astype(np.float64)
    for c in range(1, NCORES):
        out += res.results[c]["out"]
    return out.astype(np.float32).reshape(b, s, d)
